# revision 12
# baseline (speedup 1.0000x reference)
"""Trainium2 Bass kernel for nn_Attention_37056977830181.

Head-sharded (tensor-parallel) multi-head attention over 8 NeuronCores:
each core computes 2 of the 16 heads end-to-end (QKV projection, per-head
RMSNorm, softmax attention, output-projection partial sum); the host sums
the 8 partial projection outputs.

Layout strategy (all big matmuls in bf16, fp32 accumulation):
  - x is pre-transposed on the host to xT [B, C, N] so the QKV matmuls
    contract over C on the partition dimension.
  - q, k are produced directly in transposed form qT/kT [dd=128, N]
    (lhsT = W chunk, rhs = xT chunk), so scoresT needs no on-chip
    transposes. RMSNorm runs in this layout: per-head partition sums via a
    ones-block matmul; rsqrt(var) is computed as exp(-0.5*ln(var)) so the
    whole kernel stays in the single natural_log_exp_and_others ACT table
    set (no per-batch table switches, no slow DVE reciprocal).
  - v is produced in natural layout [N, dd] with a ones-column appended so
    the attention matmul's 65th output row accumulates the softmax
    denominator.
  - scoresT [j, i] per head; softmax needs no max-subtraction (scores are
    bounded ~N(0,1) after RMSNorm); exp on ACT reads PSUM directly and
    emits bf16 p-tiles; normalization is folded in after attn@v.
  - attention accumulators are evacuated to SBUF immediately (two fp32
    copies) so the next i-quarter's attn@v can start; the reciprocal /
    broadcast / normalize chain runs off the critical path with the fast
    custom-DVE reciprocal.
  - software pipelining: batch b+1's prep work (xT loads, QKV projection,
    RMSNorm, V tiles) is emitted in slices between batch b's attention
    i-quarters, so the ACT engine (the pacing engine: 64 x 1024-wide exps
    per batch) never idles at batch boundaries.
  - y partials are written in bf16 (halves HBM write traffic); the host
    accumulates the 8 partials in fp32.
"""

import dataclasses
import numpy as np
import ml_dtypes

B, N, C = 4, 2048, 1024
H = 16
D = C // H
SCALE = D**-0.5
EPS = 1e-6
N_CORES = 8
HPC = H // N_CORES  # heads per core = 2
DD = HPC * D  # per-core channel block = 128

bf16 = ml_dtypes.bfloat16

_COMPILED = {}


def _row_bcast(ap, rows):
    """View a [1, F] SBUF AP as [1, rows, F] with a 0-step middle dim, so a
    DMA with a [rows, F] destination replicates the row across partitions."""
    f = ap.shape[-1]
    (pstep, pcount), (estep, ecount) = ap.ap[0], ap.ap[-1]
    assert pcount == 1 and ecount == f
    return dataclasses.replace(ap, ap=[[pstep, 1], [0, rows], [estep, f]])


def _col_blocks(ap, starts, width):
    """View equally-spaced equal-width column blocks of a 2D AP as one AP."""
    stride = starts[1] - starts[0] if len(starts) > 1 else width
    sub = ap[:, starts[0] : starts[0] + width]
    (pstep, pcount), (estep, ecount) = sub.ap[0], sub.ap[-1]
    return dataclasses.replace(
        sub, ap=[[pstep, pcount], [estep * stride, len(starts)], [estep, width]]
    )


_WAIT_CAPS = {}
_WAIT_SKIP = {"EventSemaphore", "Call", "ISA", "UnconditionalBranch"}
_WAIT_DEFAULT_CAP = 1
_NOP_CAP = 1


def _split_waits(nc):
    """Walrus's per-instruction-struct sync-wait slots are limited (e.g. the
    self-loading-weights matmul struct takes 1, ACTIVATE takes 2).  Move
    excess waits onto no-op instructions inserted just before, on the same
    engine, preserving execution order semantics."""
    import concourse.mybir as mybir

    nid = [0]
    for f in nc.m.functions:
        for bb in f.blocks:
            out = []
            for inst in bb.instructions:
                si = inst.sync_info
                waits = list(si.on_wait) if si is not None and si.on_wait else []
                cap = (
                    10**9
                    if inst.opcode in _WAIT_SKIP
                    else _WAIT_CAPS.get(inst.opcode, _WAIT_DEFAULT_CAP)
                )
                if len(waits) > cap:
                    excess = waits[: len(waits) - cap]
                    keep = waits[len(waits) - cap :]
                    for j in range(0, len(excess), _NOP_CAP):
                        nop = mybir.InstNoOp(
                            name=f"I-waitsplit-{nid[0]}", ins=[], outs=[]
                        )
                        nid[0] += 1
                        nop.engine = inst.engine
                        nop.bass_nofuse = True
                        nop.sync_info = mybir.SyncInfo(
                            on_wait=excess[j : j + _NOP_CAP], on_update=[]
                        )
                        out.append(nop)
                    inst.sync_info = mybir.SyncInfo(
                        on_wait=keep, on_update=list(si.on_update or [])
                    )
                out.append(inst)
            bb.instructions[:] = out


def build_program(reps=1, hw_loop=0):
    """reps: python-unrolled repetitions.  hw_loop: if >0, wrap the body in a
    Tile For_i hardware loop of that many iterations (for timing runs)."""
    import contextlib
    import concourse.bass as bass
    import concourse.mybir as mybir
    import concourse.tile as tile

    F32 = mybir.dt.float32
    BF16 = mybir.dt.bfloat16
    AF = mybir.ActivationFunctionType
    MUL = mybir.AluOpType.mult

    nc = bass.Bass(
        "TRN2",
        target_bir_lowering=False,
        debug=False,
        enable_asserts=True,
        num_devices=N_CORES,
    )

    xt_d = nc.dram_tensor("xt", [B, C, N], BF16, kind="ExternalInput").ap()
    wq_d = nc.dram_tensor("wq", [128, 1024], BF16, kind="ExternalInput").ap()
    wk_d = nc.dram_tensor("wk", [128, 1024], BF16, kind="ExternalInput").ap()
    wv_d = nc.dram_tensor("wv", [128, 1024], BF16, kind="ExternalInput").ap()
    pw_d = nc.dram_tensor("pw", [DD, C], BF16, kind="ExternalInput").ap()
    qw_d = nc.dram_tensor("qw", [128, 1], F32, kind="ExternalInput").ap()
    kw_d = nc.dram_tensor("kw", [128, 1], F32, kind="ExternalInput").ap()
    onesrep_d = nc.dram_tensor("onesrep", [128, 128], BF16, kind="ExternalInput").ap()
    y_d = nc.dram_tensor("y", [B, N, C], BF16, kind="ExternalOutput").ap()

    NKC = C // 128  # 8 contraction chunks
    NJC = N // 128  # 16 key chunks
    NIH = 4  # i-quarters of 512
    IW = N // NIH  # 512

    with tile.TileContext(nc) as tc:
        with (
            tc.tile_pool(name="const", bufs=1) as cpool,
            tc.tile_pool(name="xt", bufs=2) as xpool,
            tc.tile_pool(name="qk", bufs=2) as qkpool,
            tc.tile_pool(name="v", bufs=2) as vpool,
            tc.tile_pool(name="work", bufs=2) as wpool,
            tc.tile_pool(name="y", bufs=3) as ypool,
            tc.tile_pool(name="ps", bufs=1, space="PSUM") as ps,
        ):
            # --- constants ---
            w_sb = {}
            for name, dram in (("wq", wq_d), ("wk", wk_d), ("wv", wv_d)):
                t = cpool.tile([128, 1024], BF16, tag=f"c_{name}")
                nc.sync.dma_start(out=t[:], in_=dram)
                w_sb[name] = t
            pw_sb = cpool.tile([DD, C], BF16, tag="c_pw")
            nc.sync.dma_start(out=pw_sb[:], in_=pw_d)
            qw_sb = cpool.tile([128, 1], F32, tag="c_qw")
            nc.sync.dma_start(out=qw_sb[:], in_=qw_d)
            kw_sb = cpool.tile([128, 1], F32, tag="c_kw")
            nc.sync.dma_start(out=kw_sb[:], in_=kw_d)
            onesrep_sb = cpool.tile([128, 128], BF16, tag="c_onesrep")
            nc.sync.dma_start(out=onesrep_sb[:], in_=onesrep_d)
            eps_sb = cpool.tile([128, 1], F32, tag="c_eps")
            nc.vector.memset(eps_sb[:], EPS)

            # ---------- emission helpers (software pipeline stages) ----------

            def emit_xt_loads(b, st):
                st["xt"] = []
                for kc in range(NKC):
                    t = xpool.tile([128, N], BF16, tag=f"xt{kc}", name=f"xt{kc}")
                    nc.sync.dma_start(
                        out=t[:], in_=xt_d[b, kc * 128 : (kc + 1) * 128, :]
                    )
                    st["xt"].append(t)

            def emit_qk_alloc(st):
                st["qnT"] = qkpool.tile([128, N], BF16, tag="qnT", name="qnT")
                st["knT"] = qkpool.tile([128, N], BF16, tag="knT", name="knT")
                st["qraw"] = qkpool.tile([128, N], BF16, tag="qraw", name="qraw")
                st["kraw"] = qkpool.tile([128, N], BF16, tag="kraw", name="kraw")
                st["varq"] = qkpool.tile([128, N], F32, tag="varq", name="varq")
                st["vark"] = qkpool.tile([128, N], F32, tag="vark", name="vark")

            def emit_qk_chunk_mm(st, ti, ncq):
                """Projection matmuls of one 512-column q/k chunk + square.
                The variance matmul is emitted separately (2 jc later) so
                its dependency on the DVE square never blocks the in-order
                PE queue."""
                wkey, rawT = ("wq", st["qraw"]) if ti == 0 else ("wk", st["kraw"])
                sl = slice(ncq * 512, (ncq + 1) * 512)
                pq = ps.tile([128, 512], F32, tag="smallA", name="pq")
                for kc in range(NKC):
                    nc.tensor.matmul(
                        pq[:],
                        w_sb[wkey][:, kc * 128 : (kc + 1) * 128],
                        st["xt"][kc][:, sl],
                        start=(kc == 0),
                        stop=(kc == NKC - 1),
                    )
                # single copy releases the PSUM bank; squares from SBUF
                nc.vector.tensor_copy(rawT[:, sl], pq[:])
                sq = wpool.tile([128, 512], BF16, tag="sq", bufs=4)
                nc.vector.tensor_mul(sq[:], rawT[:, sl], rawT[:, sl])
                st[f"sq{ti}{ncq}"] = sq

            def emit_qk_chunk_var(st, ti, ncq):
                var = st["varq"] if ti == 0 else st["vark"]
                sl = slice(ncq * 512, (ncq + 1) * 512)
                psums = ps.tile([128, 512], F32, tag="smallA", name="psums")
                nc.tensor.matmul(
                    psums[:], onesrep_sb[:], st.pop(f"sq{ti}{ncq}")[:],
                    start=True, stop=True,
                )
                nc.vector.tensor_copy(var[:, sl], psums[:])

            def emit_qk_chunk(st, ti, ncq):
                emit_qk_chunk_mm(st, ti, ncq)
                emit_qk_chunk_var(st, ti, ncq)

            def emit_rsqrt(st, ti):
                """rsqrt(var/D + eps) = exp(-0.5 * ln(var/D + eps)): stays in
                one ACT table set, no DVE reciprocal."""
                var = st["varq"] if ti == 0 else st["vark"]
                nc.scalar.activation(
                    var[:], var[:], AF.Ln, bias=eps_sb[:], scale=1.0 / D
                )
                nc.scalar.activation(var[:], var[:], AF.Exp, scale=-0.5)

            def emit_stt(st, ti, ncq):
                rawT, wcol, var, dstT = (
                    (st["qraw"], qw_sb, st["varq"], st["qnT"])
                    if ti == 0
                    else (st["kraw"], kw_sb, st["vark"], st["knT"])
                )
                sl = slice(ncq * 512, (ncq + 1) * 512)
                nc.vector.scalar_tensor_tensor(
                    dstT[:, sl],
                    rawT[:, sl],
                    wcol[:],
                    var[:, sl],
                    op0=MUL,
                    op1=MUL,
                )

            def emit_v(st, jc):
                # alternate PSUM banks so pv never waits on its own
                # vt-copy's DVE round-trip
                pvt = ps.tile(
                    [128, 512], F32, tag=("smallA" if jc % 2 else "smallB"), name="pv"
                )
                pv = pvt[:, 0:128]
                for kc in range(NKC):
                    nc.tensor.matmul(
                        pv,
                        st["xt"][kc][:, jc * 128 : (jc + 1) * 128],
                        w_sb["wv"][:, kc * 128 : (kc + 1) * 128],
                        start=(kc == 0),
                        stop=(kc == NKC - 1),
                    )
                vt = vpool.tile([128, 130], BF16, tag=f"v{jc}", name=f"v{jc}")
                nc.vector.tensor_copy(
                    _col_blocks(vt[:], (0, 65), 64),
                    _col_blocks(pvt[:], (0, 64), 64),
                )
                nc.vector.memset(_col_blocks(vt[:], (64, 129), 1), 1.0)
                st["v"].append(vt)

            def emit_attention_jcloop(st, ih, inject):
                """16 jc steps of scores/exp/attn@v for i-quarter ih.
                inject maps jc -> closure emitted after that step (used to
                slot next-batch prep work and the previous quarter's
                projection into the PE stream so it never idles long
                enough for the HAM clock gate to re-throttle)."""
                qnT, knT = st["qnT"], st["knT"]
                isl = slice(ih * IW, (ih + 1) * IW)
                acc_h = []
                for h in range(HPC):
                    acc_t = ps.tile([65, IW], F32, tag=f"acc{h}", name=f"acc_t{h}")
                    acc_h.append(acc_t)
                produce_v = ih == 0
                p_prev = None
                for jc in range(NJC):
                    scs = ps.tile([128, 2 * IW], F32, tag="scs", bufs=2, name="scs")
                    for h in range(HPC):
                        hs = slice(h * 64, (h + 1) * 64)
                        nc.tensor.matmul(
                            scs[:, h * IW : (h + 1) * IW],
                            knT[hs, jc * 128 : (jc + 1) * 128],
                            qnT[hs, isl],
                            start=True,
                            stop=True,
                            tile_position=(h * 64, 0),
                        )
                    if produce_v:
                        emit_v(st, jc)
                    if p_prev is not None:
                        vt = st["v"][jc - 1]
                        for h in range(HPC):
                            nc.tensor.matmul(
                                acc_h[h][:],
                                vt[:, h * 65 : h * 65 + 65],
                                p_prev[:, h * IW : (h + 1) * IW],
                                start=(jc == 1),
                                stop=False,
                            )
                    p = wpool.tile([128, 2 * IW], BF16, tag="p", bufs=4)
                    nc.scalar.activation(p[:], scs[:], AF.Exp, scale=SCALE)
                    p_prev = p
                    if jc in inject:
                        inject[jc]()
                vt = st["v"][NJC - 1]
                for h in range(HPC):
                    nc.tensor.matmul(
                        acc_h[h][:],
                        vt[:, h * 65 : h * 65 + 65],
                        p_prev[:, h * IW : (h + 1) * IW],
                        start=False,
                        stop=True,
                    )
                return acc_h

            def emit_evac(st, ih, acc_h):
                """Evacuate accumulators to SBUF (frees the PSUM banks for
                the next i-quarter) and build the normalized outT tile.
                The softmax reciprocal runs on a DMA-transposed [128, 4]
                column so the DVE iterative divide (8 cyc/elem) touches only
                4 elements per lane instead of 512."""
                outU = []
                for h in range(HPC):
                    u = wpool.tile([65, IW], F32, tag=f"outU{h}", bufs=2)
                    # ACT is idle at the i-quarter boundary (waiting on next
                    # scores); evacuating there frees the accumulators
                    # without queueing behind DVE work
                    nc.scalar.copy(u[:], acc_h[h][:])
                    outU.append(u)
                outTn = wpool.tile([128, IW], BF16, tag="outTn", bufs=3)
                for h in range(HPC):
                    dcol = wpool.tile([128, 4], F32, tag=f"dcol{h}", bufs=2, name="dcol")
                    for m in range(4):
                        nc.sync.dma_start(
                            out=dcol[:, m : m + 1],
                            in_=outU[h][64:65, m * 128 : (m + 1) * 128],
                        )
                    rcol = wpool.tile([128, 4], F32, tag=f"rcol{h}", bufs=2, name="rcol")
                    nc.vector.reciprocal(rcol[:], dcol[:])
                    rrow = wpool.tile([1, IW], F32, tag=f"rrow{h}", bufs=2, name="rrow")
                    for m in range(4):
                        nc.sync.dma_start(
                            out=rrow[0:1, m * 128 : (m + 1) * 128],
                            in_=rcol[:, m : m + 1],
                        )
                    rb = wpool.tile([64, IW], F32, tag=f"rb{h}", bufs=2, name="rb")
                    nc.sync.dma_start(out=rb[:], in_=_row_bcast(rrow[0:1, :], 64))
                    nc.vector.tensor_mul(
                        outTn[h * 64 : (h + 1) * 64, :],
                        outU[h][0:64, :],
                        rb[:],
                    )
                return outTn

            def emit_drain_rest(st, ih, outTn):
                b = st["b"]
                for mc in range(IW // 128):
                    ysb = ypool.tile([128, C], BF16, tag="ysb")
                    for oc in range(C // 512):
                        yp = ps.tile([128, 512], F32, tag="smallB", name="yp")
                        nc.tensor.matmul(
                            yp[:],
                            outTn[:, mc * 128 : (mc + 1) * 128],
                            pw_sb[:, oc * 512 : (oc + 1) * 512],
                            start=True,
                            stop=True,
                        )
                        nc.vector.tensor_copy(ysb[:, oc * 512 : (oc + 1) * 512], yp[:])
                    qi0 = ih * IW + mc * 128
                    nc.sync.dma_start(out=y_d[b, qi0 : qi0 + 128, :], in_=ysb[:])

            def chain(*fns):
                fns = [f for f in fns if f is not None]
                def run():
                    for f in fns:
                        f()
                return run

            def prep_injections(nxt, ih):
                """Next-batch prep work slotted into this i-quarter's jc
                loop, one small piece per point, scheduled so every PE
                piece's DVE inputs are ready ~2 jc before the PE reaches
                it."""
                if nxt is None:
                    return {}
                if ih == 0:
                    return {
                        2: chain(lambda: emit_xt_loads(nxt["b"], nxt),
                                 lambda: emit_qk_alloc(nxt)),
                        6: lambda: emit_qk_chunk_mm(nxt, 1, 0),
                        8: lambda: emit_qk_chunk_var(nxt, 1, 0),
                        10: lambda: emit_qk_chunk_mm(nxt, 1, 1),
                        12: lambda: emit_qk_chunk_var(nxt, 1, 1),
                        14: lambda: emit_qk_chunk_mm(nxt, 1, 2),
                    }
                if ih == 1:
                    return {
                        1: lambda: emit_qk_chunk_var(nxt, 1, 2),
                        3: lambda: emit_qk_chunk_mm(nxt, 1, 3),
                        5: lambda: emit_qk_chunk_var(nxt, 1, 3),
                        8: lambda: nc.scalar.activation(
                            nxt["vark"][:], nxt["vark"][:], AF.Ln,
                            bias=eps_sb[:], scale=1.0 / D),
                        11: lambda: nc.scalar.activation(
                            nxt["vark"][:], nxt["vark"][:], AF.Exp, scale=-0.5),
                        13: chain(lambda: emit_stt(nxt, 1, 0),
                                  lambda: emit_stt(nxt, 1, 1)),
                        15: chain(lambda: emit_stt(nxt, 1, 2),
                                  lambda: emit_stt(nxt, 1, 3)),
                    }
                if ih == 2:
                    return {
                        1: lambda: emit_qk_chunk_mm(nxt, 0, 0),
                        3: lambda: emit_qk_chunk_var(nxt, 0, 0),
                        5: lambda: emit_qk_chunk_mm(nxt, 0, 1),
                        7: lambda: emit_qk_chunk_var(nxt, 0, 1),
                        9: lambda: emit_qk_chunk_mm(nxt, 0, 2),
                        11: lambda: emit_qk_chunk_var(nxt, 0, 2),
                        13: lambda: emit_qk_chunk_mm(nxt, 0, 3),
                        15: lambda: emit_qk_chunk_var(nxt, 0, 3),
                    }
                return {
                    2: lambda: nc.scalar.activation(
                        nxt["varq"][:], nxt["varq"][:], AF.Ln,
                        bias=eps_sb[:], scale=1.0 / D),
                    5: lambda: nc.scalar.activation(
                        nxt["varq"][:], nxt["varq"][:], AF.Exp, scale=-0.5),
                    8: chain(lambda: emit_stt(nxt, 0, 0),
                             lambda: emit_stt(nxt, 0, 1)),
                    10: chain(lambda: emit_stt(nxt, 0, 2),
                              lambda: emit_stt(nxt, 0, 3)),
                }

            # ---------- pipelined batch loop ----------

            loop_ctx = tc.For_i(0, hw_loop, 1) if hw_loop else contextlib.nullcontext()
            with loop_ctx:
              for rep in range(reps):
                # PE pre-warm: a burst of tiny matmuls during the xT load
                # trips the HAM un-throttle (~3.4us of activity) so the
                # prologue projection runs at full clock
                st = {"b": 0, "v": []}
                emit_xt_loads(0, st)
                emit_qk_alloc(st)
                warm = ps.tile([128, 64], F32, tag="smallB", name="warm")
                for w in range(48):
                    nc.tensor.matmul(
                        warm[:], onesrep_sb[:], onesrep_sb[:, 0:64],
                        start=(w == 0), stop=(w == 47),
                    )
                for ncq in range(4):
                    emit_qk_chunk(st, 1, ncq)
                emit_rsqrt(st, 1)
                for ncq in range(4):
                    emit_stt(st, 1, ncq)
                for ncq in range(4):
                    emit_qk_chunk(st, 0, ncq)
                emit_rsqrt(st, 0)
                for ncq in range(4):
                    emit_stt(st, 0, ncq)
                pendings = []  # deferred per-i-quarter projection closures
                for b in range(B):
                    nxt = {"b": b + 1, "v": []} if b + 1 < B else None
                    for ih in range(NIH):
                        inj = prep_injections(nxt, ih)
                        if ih > 0:
                            # drain-free i-quarter 0 keeps smallB clear for
                            # the alternating-bank V production
                            for slot in (4, 9):
                                if pendings:
                                    p0 = pendings.pop(0)
                                    inj[slot] = chain(inj.get(slot), p0)
                                if ih != 1:
                                    break
                        acc_h = emit_attention_jcloop(st, ih, inj)
                        outTn = emit_evac(st, ih, acc_h)
                        pendings.append(
                            lambda st=st, ih=ih, outTn=outTn: emit_drain_rest(st, ih, outTn)
                        )
                    st = nxt
                for p0 in pendings:
                    p0()
    _split_waits(nc)
    return nc


def _prepare_inputs(x, qkv_w, q_norm_w, k_norm_w, proj_w):
    """Host-side sharding/layout prep. Returns per-core input maps."""
    xt = np.ascontiguousarray(x.transpose(0, 2, 1)).astype(bf16)  # [B, C, N]
    qw_col = np.tile(q_norm_w, HPC).reshape(128, 1).astype(np.float32)
    kw_col = np.tile(k_norm_w, HPC).reshape(128, 1).astype(np.float32)
    onesrep = np.zeros((128, 128), bf16)
    onesrep[0:64, 0:64] = 1
    onesrep[64:128, 64:128] = 1

    in_maps = []
    for c in range(N_CORES):
        rows = slice(DD * c, DD * (c + 1))

        def pack(w):  # [128 rows, C] -> packed lhsT chunks [128, 1024]
            chunks = [
                np.ascontiguousarray(w[:, kc * 128 : (kc + 1) * 128].T)
                for kc in range(C // 128)
            ]
            return np.concatenate(chunks, axis=1).astype(bf16)

        in_maps.append(
            {
                "xt": xt,
                "wq": pack(qkv_w[0 * C :][rows, :]),
                "wk": pack(qkv_w[1 * C + DD * c : 1 * C + DD * (c + 1), :]),
                "wv": pack(qkv_w[2 * C + DD * c : 2 * C + DD * (c + 1), :]),
                "pw": np.ascontiguousarray(proj_w[:, rows].T).astype(bf16),
                "qw": qw_col,
                "kw": kw_col,
                "onesrep": onesrep,
            }
        )
    return in_maps


def run_on_device(in_maps, reps=1, hw_loop=0):
    from concourse.bass_utils import run_bass_kernel_spmd

    key = (reps, hw_loop)
    if key not in _COMPILED:
        _COMPILED[key] = build_program(reps, hw_loop=hw_loop)
    nc = _COMPILED[key]
    res = run_bass_kernel_spmd(nc, in_maps, list(range(N_CORES)))
    return res


def kernel(x, qkv_w, q_norm_w, k_norm_w, proj_w, proj_b):
    x = np.asarray(x, np.float32)
    qkv_w = np.asarray(qkv_w, np.float32)
    proj_w = np.asarray(proj_w, np.float32)
    in_maps = _prepare_inputs(
        x, qkv_w, np.asarray(q_norm_w, np.float32), np.asarray(k_norm_w, np.float32), proj_w
    )
    res = run_on_device(in_maps, reps=1)
    y = np.zeros((B, N, C), np.float32)
    for c in range(N_CORES):
        y += np.asarray(res.results[c]["y"], dtype=np.float32)
    y += np.asarray(proj_b, np.float32)[None, None, :]
    return y


# revision 15
# speedup vs baseline: 1.1377x; 1.1377x over previous
"""Trainium2 Bass kernel for nn_Attention_37056977830181.

Head-sharded (tensor-parallel) multi-head attention over 8 NeuronCores:
each core computes 2 of the 16 heads end-to-end (QKV projection, per-head
RMSNorm, softmax attention, output-projection partial sum); the host sums
the 8 partial projection outputs.

Layout strategy (all big matmuls in bf16, fp32 accumulation):
  - x is pre-transposed on the host to xT [B, C, N] so the QKV matmuls
    contract over C on the partition dimension.
  - q, k are produced directly in transposed form qT/kT [dd=128, N]
    (lhsT = W chunk, rhs = xT chunk), so scoresT needs no on-chip
    transposes. RMSNorm runs in this layout: per-head partition sums via a
    ones-block matmul; rsqrt(var) is computed as exp(-0.5*ln(var)) so the
    whole kernel stays in the single natural_log_exp_and_others ACT table
    set (no per-batch table switches, no slow DVE reciprocal).
  - v is produced in natural layout [N, dd] with a ones-column appended so
    the attention matmul's 65th output row accumulates the softmax
    denominator.
  - scoresT [j, i] per head; softmax needs no max-subtraction (scores are
    bounded ~N(0,1) after RMSNorm); exp on ACT reads PSUM directly and
    emits bf16 p-tiles; normalization is folded in after attn@v.
  - attention accumulators are evacuated to SBUF immediately (two fp32
    copies) so the next i-quarter's attn@v can start; the reciprocal /
    broadcast / normalize chain runs off the critical path with the fast
    custom-DVE reciprocal.
  - software pipelining: batch b+1's prep work (xT loads, QKV projection,
    RMSNorm, V tiles) is emitted in slices between batch b's attention
    i-quarters, so the ACT engine (the pacing engine: 64 x 1024-wide exps
    per batch) never idles at batch boundaries.
  - y partials are written in bf16 (halves HBM write traffic); the host
    accumulates the 8 partials in fp32.
"""

import dataclasses
import numpy as np
import ml_dtypes

B, N, C = 4, 2048, 1024
H = 16
D = C // H
SCALE = D**-0.5
EPS = 1e-6
N_CORES = 8
HPC = H // N_CORES  # heads per core = 2
DD = HPC * D  # per-core channel block = 128

bf16 = ml_dtypes.bfloat16

_COMPILED = {}


def _row_bcast(ap, rows):
    """View a [1, F] SBUF AP as [1, rows, F] with a 0-step middle dim, so a
    DMA with a [rows, F] destination replicates the row across partitions."""
    f = ap.shape[-1]
    (pstep, pcount), (estep, ecount) = ap.ap[0], ap.ap[-1]
    assert pcount == 1 and ecount == f
    return dataclasses.replace(ap, ap=[[pstep, 1], [0, rows], [estep, f]])


def _col_blocks(ap, starts, width):
    """View equally-spaced equal-width column blocks of a 2D AP as one AP."""
    stride = starts[1] - starts[0] if len(starts) > 1 else width
    sub = ap[:, starts[0] : starts[0] + width]
    (pstep, pcount), (estep, ecount) = sub.ap[0], sub.ap[-1]
    return dataclasses.replace(
        sub, ap=[[pstep, pcount], [estep * stride, len(starts)], [estep, width]]
    )


_WAIT_CAPS = {}
_WAIT_SKIP = {"EventSemaphore", "Call", "ISA", "UnconditionalBranch"}
_WAIT_DEFAULT_CAP = 1
_NOP_CAP = 1


def _split_waits(nc):
    """Walrus's per-instruction-struct sync-wait slots are limited (e.g. the
    self-loading-weights matmul struct takes 1, ACTIVATE takes 2).  Move
    excess waits onto no-op instructions inserted just before, on the same
    engine, preserving execution order semantics."""
    import concourse.mybir as mybir

    nid = [0]
    for f in nc.m.functions:
        for bb in f.blocks:
            out = []
            for inst in bb.instructions:
                si = inst.sync_info
                waits = list(si.on_wait) if si is not None and si.on_wait else []
                cap = (
                    10**9
                    if inst.opcode in _WAIT_SKIP
                    else _WAIT_CAPS.get(inst.opcode, _WAIT_DEFAULT_CAP)
                )
                if len(waits) > cap:
                    excess = waits[: len(waits) - cap]
                    keep = waits[len(waits) - cap :]
                    for j in range(0, len(excess), _NOP_CAP):
                        nop = mybir.InstNoOp(
                            name=f"I-waitsplit-{nid[0]}", ins=[], outs=[]
                        )
                        nid[0] += 1
                        nop.engine = inst.engine
                        nop.bass_nofuse = True
                        nop.sync_info = mybir.SyncInfo(
                            on_wait=excess[j : j + _NOP_CAP], on_update=[]
                        )
                        out.append(nop)
                    inst.sync_info = mybir.SyncInfo(
                        on_wait=keep, on_update=list(si.on_update or [])
                    )
                out.append(inst)
            bb.instructions[:] = out


def build_program(reps=1, hw_loop=0):
    """reps: python-unrolled repetitions.  hw_loop: if >0, wrap the body in a
    Tile For_i hardware loop of that many iterations (for timing runs)."""
    import contextlib
    import concourse.bass as bass
    import concourse.mybir as mybir
    import concourse.tile as tile

    F32 = mybir.dt.float32
    BF16 = mybir.dt.bfloat16
    AF = mybir.ActivationFunctionType
    MUL = mybir.AluOpType.mult

    nc = bass.Bass(
        "TRN2",
        target_bir_lowering=False,
        debug=False,
        enable_asserts=True,
        num_devices=N_CORES,
    )

    xt_d = nc.dram_tensor("xt", [B, C, N], BF16, kind="ExternalInput").ap()
    wq_d = nc.dram_tensor("wq", [128, 1024], BF16, kind="ExternalInput").ap()
    wk_d = nc.dram_tensor("wk", [128, 1024], BF16, kind="ExternalInput").ap()
    wv_d = nc.dram_tensor("wv", [128, 1024], BF16, kind="ExternalInput").ap()
    pw_d = nc.dram_tensor("pw", [DD, C], BF16, kind="ExternalInput").ap()
    qw_d = nc.dram_tensor("qw", [128, 1], F32, kind="ExternalInput").ap()
    kw_d = nc.dram_tensor("kw", [128, 1], F32, kind="ExternalInput").ap()
    onesrep_d = nc.dram_tensor("onesrep", [128, 128], BF16, kind="ExternalInput").ap()
    y_d = nc.dram_tensor("y", [B, N, C], BF16, kind="ExternalOutput").ap()

    NKC = C // 128  # 8 contraction chunks
    NJC = N // 128  # 16 key chunks
    NIH = 4  # i-quarters of 512
    IW = N // NIH  # 512

    with tile.TileContext(nc) as tc:
        with (
            tc.tile_pool(name="const", bufs=1) as cpool,
            tc.tile_pool(name="xt", bufs=2) as xpool,
            tc.tile_pool(name="qk", bufs=2) as qkpool,
            tc.tile_pool(name="v", bufs=2) as vpool,
            tc.tile_pool(name="work", bufs=2) as wpool,
            tc.tile_pool(name="y", bufs=3) as ypool,
            tc.tile_pool(name="ps", bufs=1, space="PSUM") as ps,
        ):
            # --- constants ---
            w_sb = {}
            for name, dram in (("wq", wq_d), ("wk", wk_d), ("wv", wv_d)):
                t = cpool.tile([128, 1024], BF16, tag=f"c_{name}")
                nc.sync.dma_start(out=t[:], in_=dram)
                w_sb[name] = t
            pw_sb = cpool.tile([DD, C], BF16, tag="c_pw")
            nc.sync.dma_start(out=pw_sb[:], in_=pw_d)
            qw_sb = cpool.tile([128, 1], F32, tag="c_qw")
            nc.sync.dma_start(out=qw_sb[:], in_=qw_d)
            kw_sb = cpool.tile([128, 1], F32, tag="c_kw")
            nc.sync.dma_start(out=kw_sb[:], in_=kw_d)
            onesrep_sb = cpool.tile([128, 128], BF16, tag="c_onesrep")
            nc.sync.dma_start(out=onesrep_sb[:], in_=onesrep_d)
            eps_sb = cpool.tile([128, 1], F32, tag="c_eps")
            nc.vector.memset(eps_sb[:], EPS)

            # ---------- emission helpers (software pipeline stages) ----------

            def emit_xt_loads(b, st):
                st["xt"] = []
                for kc in range(NKC):
                    t = xpool.tile([128, N], BF16, tag=f"xt{kc}", name=f"xt{kc}")
                    nc.sync.dma_start(
                        out=t[:], in_=xt_d[b, kc * 128 : (kc + 1) * 128, :]
                    )
                    st["xt"].append(t)

            def emit_qk_alloc(st):
                st["qnT"] = qkpool.tile([128, N], BF16, tag="qnT", name="qnT")
                st["knT"] = qkpool.tile([128, N], BF16, tag="knT", name="knT")
                st["qraw"] = qkpool.tile([128, N], BF16, tag="qraw", name="qraw")
                st["kraw"] = qkpool.tile([128, N], BF16, tag="kraw", name="kraw")
                st["varq"] = qkpool.tile([128, N], F32, tag="varq", name="varq")
                st["vark"] = qkpool.tile([128, N], F32, tag="vark", name="vark")

            def emit_qk_chunk_mm(st, ti, ncq):
                """Projection matmuls of one 512-column q/k chunk + square.
                The variance matmul is emitted separately (a few jc later)
                so its dependency on the DVE square never blocks the
                in-order PE queue."""
                wkey, rawT = ("wq", st["qraw"]) if ti == 0 else ("wk", st["kraw"])
                sl = slice(ncq * 512, (ncq + 1) * 512)
                pq = ps.tile([128, 512], F32, tag="smallA", name="pq")
                for kc in range(NKC):
                    nc.tensor.matmul(
                        pq[:],
                        w_sb[wkey][:, kc * 128 : (kc + 1) * 128],
                        st["xt"][kc][:, sl],
                        start=(kc == 0),
                        stop=(kc == NKC - 1),
                    )
                # single copy releases the PSUM bank; squares from SBUF
                nc.vector.tensor_copy(rawT[:, sl], pq[:])
                sq = wpool.tile([128, 512], BF16, tag="sq", bufs=4)
                nc.vector.tensor_mul(sq[:], rawT[:, sl], rawT[:, sl])
                st[f"sq{ti}{ncq}"] = sq

            def emit_qk_chunk_var(st, ti, ncq):
                var = st["varq"] if ti == 0 else st["vark"]
                sl = slice(ncq * 512, (ncq + 1) * 512)
                psums = ps.tile([128, 512], F32, tag="smallA", name="psums")
                nc.tensor.matmul(
                    psums[:], onesrep_sb[:], st.pop(f"sq{ti}{ncq}")[:],
                    start=True, stop=True,
                )
                nc.vector.tensor_copy(var[:, sl], psums[:])

            def emit_qk_chunk(st, ti, ncq):
                emit_qk_chunk_mm(st, ti, ncq)
                emit_qk_chunk_var(st, ti, ncq)

            def emit_rsqrt(st, ti):
                """rsqrt(var/D + eps) = exp(-0.5 * ln(var/D + eps)): stays in
                one ACT table set, no DVE reciprocal."""
                var = st["varq"] if ti == 0 else st["vark"]
                nc.scalar.activation(
                    var[:], var[:], AF.Ln, bias=eps_sb[:], scale=1.0 / D
                )
                nc.scalar.activation(var[:], var[:], AF.Exp, scale=-0.5)

            def emit_stt(st, ti, ncq):
                rawT, wcol, var, dstT = (
                    (st["qraw"], qw_sb, st["varq"], st["qnT"])
                    if ti == 0
                    else (st["kraw"], kw_sb, st["vark"], st["knT"])
                )
                sl = slice(ncq * 512, (ncq + 1) * 512)
                nc.vector.scalar_tensor_tensor(
                    dstT[:, sl],
                    rawT[:, sl],
                    wcol[:],
                    var[:, sl],
                    op0=MUL,
                    op1=MUL,
                )

            def emit_v(st, jc):
                # alternate PSUM banks so pv never waits on its own
                # vt-copy's DVE round-trip
                pvt = ps.tile(
                    [128, 512], F32, tag=("smallA" if jc % 2 else "smallB"), name="pv"
                )
                pv = pvt[:, 0:128]
                for kc in range(NKC):
                    nc.tensor.matmul(
                        pv,
                        st["xt"][kc][:, jc * 128 : (jc + 1) * 128],
                        w_sb["wv"][:, kc * 128 : (kc + 1) * 128],
                        start=(kc == 0),
                        stop=(kc == NKC - 1),
                    )
                vt = vpool.tile([128, 130], BF16, tag=f"v{jc}", name=f"v{jc}")
                nc.vector.tensor_copy(
                    _col_blocks(vt[:], (0, 65), 64),
                    _col_blocks(pvt[:], (0, 64), 64),
                )
                nc.vector.memset(_col_blocks(vt[:], (64, 129), 1), 1.0)
                st["v"].append(vt)

            def emit_attention_jcloop(st, ih, inject):
                """16 jc steps of scores/exp/attn@v for i-quarter ih.
                inject maps jc -> closure emitted after that step (used to
                slot next-batch prep work and the previous quarter's
                projection into the PE stream so it never idles long
                enough for the HAM clock gate to re-throttle)."""
                qnT, knT = st["qnT"], st["knT"]
                isl = slice(ih * IW, (ih + 1) * IW)
                acc_h = []
                for h in range(HPC):
                    acc_t = ps.tile([65, IW], F32, tag=f"acc{h}", name=f"acc_t{h}")
                    acc_h.append(acc_t)
                produce_v = ih == 0
                p_prev = None
                for jc in range(NJC):
                    scs = ps.tile([128, 2 * IW], F32, tag="scs", bufs=2, name="scs")
                    for h in range(HPC):
                        hs = slice(h * 64, (h + 1) * 64)
                        nc.tensor.matmul(
                            scs[:, h * IW : (h + 1) * IW],
                            knT[hs, jc * 128 : (jc + 1) * 128],
                            qnT[hs, isl],
                            start=True,
                            stop=True,
                            tile_position=(h * 64, 0),
                        )
                    if produce_v:
                        emit_v(st, jc)
                    if p_prev is not None:
                        vt = st["v"][jc - 1]
                        for h in range(HPC):
                            nc.tensor.matmul(
                                acc_h[h][:],
                                vt[:, h * 65 : h * 65 + 65],
                                p_prev[:, h * IW : (h + 1) * IW],
                                start=(jc == 1),
                                stop=False,
                            )
                    p = wpool.tile([128, 2 * IW], BF16, tag="p", bufs=4)
                    nc.scalar.activation(p[:], scs[:], AF.Exp, scale=SCALE)
                    p_prev = p
                    if jc in inject:
                        inject[jc]()
                vt = st["v"][NJC - 1]
                for h in range(HPC):
                    nc.tensor.matmul(
                        acc_h[h][:],
                        vt[:, h * 65 : h * 65 + 65],
                        p_prev[:, h * IW : (h + 1) * IW],
                        start=False,
                        stop=True,
                    )
                return acc_h

            def emit_evac(st, ih, acc_h):
                """Evacuate accumulators to SBUF (frees the PSUM banks for
                the next i-quarter) and build the normalized outT tile.
                The softmax reciprocal runs on a DMA-transposed [128, 4]
                column so the DVE iterative divide (8 cyc/elem) touches only
                4 elements per lane instead of 512."""
                outU = []
                for h in range(HPC):
                    u = wpool.tile([65, IW], F32, tag=f"outU{h}", bufs=2)
                    nc.vector.tensor_copy(u[:], acc_h[h][:])
                    outU.append(u)
                outTn = wpool.tile([128, IW], BF16, tag="outTn", bufs=3)
                for h in range(HPC):
                    dcol = wpool.tile([128, 4], F32, tag=f"dcol{h}", bufs=2, name="dcol")
                    for m in range(4):
                        nc.sync.dma_start(
                            out=dcol[:, m : m + 1],
                            in_=outU[h][64:65, m * 128 : (m + 1) * 128],
                        )
                    rcol = wpool.tile([128, 4], F32, tag=f"rcol{h}", bufs=2, name="rcol")
                    nc.vector.reciprocal(rcol[:], dcol[:])
                    rrow = wpool.tile([1, IW], F32, tag=f"rrow{h}", bufs=2, name="rrow")
                    for m in range(4):
                        nc.sync.dma_start(
                            out=rrow[0:1, m * 128 : (m + 1) * 128],
                            in_=rcol[:, m : m + 1],
                        )
                    rb = wpool.tile([64, IW], F32, tag=f"rb{h}", bufs=2, name="rb")
                    nc.sync.dma_start(out=rb[:], in_=_row_bcast(rrow[0:1, :], 64))
                    nc.vector.tensor_mul(
                        outTn[h * 64 : (h + 1) * 64, :],
                        outU[h][0:64, :],
                        rb[:],
                    )
                return outTn

            def emit_drain_rest(st, ih, outTn):
                b = st["b"]
                for mc in range(IW // 128):
                    ysb = ypool.tile([128, C], BF16, tag="ysb")
                    for oc in range(C // 512):
                        yp = ps.tile([128, 512], F32, tag="smallB", name="yp")
                        nc.tensor.matmul(
                            yp[:],
                            outTn[:, mc * 128 : (mc + 1) * 128],
                            pw_sb[:, oc * 512 : (oc + 1) * 512],
                            start=True,
                            stop=True,
                        )
                        nc.vector.tensor_copy(ysb[:, oc * 512 : (oc + 1) * 512], yp[:])
                    qi0 = ih * IW + mc * 128
                    nc.sync.dma_start(out=y_d[b, qi0 : qi0 + 128, :], in_=ysb[:])

            def chain(*fns):
                fns = [f for f in fns if f is not None]
                def run():
                    for f in fns:
                        f()
                return run

            def prep_injections(nxt, ih):
                """Next-batch prep work slotted into this i-quarter's jc
                loop, one small piece per point, scheduled so every PE
                piece's DVE inputs are ready a couple of jc before the PE
                reaches it."""
                if nxt is None:
                    return {}
                if ih == 0:
                    return {
                        2: chain(lambda: emit_xt_loads(nxt["b"], nxt),
                                 lambda: emit_qk_alloc(nxt)),
                        6: lambda: emit_qk_chunk_mm(nxt, 1, 0),
                        8: lambda: emit_qk_chunk_var(nxt, 1, 0),
                        10: lambda: emit_qk_chunk_mm(nxt, 1, 1),
                        12: lambda: emit_qk_chunk_var(nxt, 1, 1),
                        14: lambda: emit_qk_chunk_mm(nxt, 1, 2),
                    }
                if ih == 1:
                    return {
                        1: lambda: emit_qk_chunk_var(nxt, 1, 2),
                        3: lambda: emit_qk_chunk_mm(nxt, 1, 3),
                        5: lambda: emit_qk_chunk_var(nxt, 1, 3),
                        7: lambda: nc.scalar.activation(
                            nxt["vark"][:], nxt["vark"][:], AF.Ln,
                            bias=eps_sb[:], scale=1.0 / D),
                        10: lambda: nc.scalar.activation(
                            nxt["vark"][:], nxt["vark"][:], AF.Exp, scale=-0.5),
                        12: chain(lambda: emit_stt(nxt, 1, 0),
                                  lambda: emit_stt(nxt, 1, 1)),
                        14: chain(lambda: emit_stt(nxt, 1, 2),
                                  lambda: emit_stt(nxt, 1, 3)),
                    }
                if ih == 2:
                    return {
                        1: lambda: emit_qk_chunk_mm(nxt, 0, 0),
                        3: lambda: emit_qk_chunk_var(nxt, 0, 0),
                        5: lambda: emit_qk_chunk_mm(nxt, 0, 1),
                        7: lambda: emit_qk_chunk_var(nxt, 0, 1),
                        9: lambda: emit_qk_chunk_mm(nxt, 0, 2),
                        11: lambda: emit_qk_chunk_var(nxt, 0, 2),
                        13: lambda: emit_qk_chunk_mm(nxt, 0, 3),
                        15: lambda: emit_qk_chunk_var(nxt, 0, 3),
                    }
                return {
                    2: lambda: nc.scalar.activation(
                        nxt["varq"][:], nxt["varq"][:], AF.Ln,
                        bias=eps_sb[:], scale=1.0 / D),
                    5: lambda: nc.scalar.activation(
                        nxt["varq"][:], nxt["varq"][:], AF.Exp, scale=-0.5),
                    8: chain(lambda: emit_stt(nxt, 0, 0),
                             lambda: emit_stt(nxt, 0, 1)),
                    10: chain(lambda: emit_stt(nxt, 0, 2),
                              lambda: emit_stt(nxt, 0, 3)),
                }

            # ---------- pipelined batch loop ----------

            loop_ctx = tc.For_i(0, hw_loop, 1) if hw_loop else contextlib.nullcontext()
            with loop_ctx:
              for rep in range(reps):
                # PE pre-warm: a burst of tiny matmuls during the xT load
                # trips the HAM un-throttle (~3.4us of activity) so the
                # prologue projection runs at full clock
                st = {"b": 0, "v": []}
                emit_xt_loads(0, st)
                emit_qk_alloc(st)
                for ncq in range(4):
                    emit_qk_chunk(st, 1, ncq)
                emit_rsqrt(st, 1)
                for ncq in range(4):
                    emit_stt(st, 1, ncq)
                for ncq in range(4):
                    emit_qk_chunk(st, 0, ncq)
                emit_rsqrt(st, 0)
                for ncq in range(4):
                    emit_stt(st, 0, ncq)
                pendings = []  # deferred per-i-quarter projection closures
                for b in range(B):
                    nxt = {"b": b + 1, "v": []} if b + 1 < B else None
                    for ih in range(NIH):
                        inj = prep_injections(nxt, ih)
                        if ih > 0:
                            # drain-free i-quarter 0 keeps smallB clear for
                            # the alternating-bank V production
                            for slot in (4, 9):
                                if pendings:
                                    p0 = pendings.pop(0)
                                    inj[slot] = chain(inj.get(slot), p0)
                                if ih != 1:
                                    break
                        acc_h = emit_attention_jcloop(st, ih, inj)
                        outTn = emit_evac(st, ih, acc_h)
                        pendings.append(
                            lambda st=st, ih=ih, outTn=outTn: emit_drain_rest(st, ih, outTn)
                        )
                    st = nxt
                for p0 in pendings:
                    p0()
    _split_waits(nc)
    return nc


def _prepare_inputs(x, qkv_w, q_norm_w, k_norm_w, proj_w):
    """Host-side sharding/layout prep. Returns per-core input maps."""
    xt = np.ascontiguousarray(x.transpose(0, 2, 1)).astype(bf16)  # [B, C, N]
    qw_col = np.tile(q_norm_w, HPC).reshape(128, 1).astype(np.float32)
    kw_col = np.tile(k_norm_w, HPC).reshape(128, 1).astype(np.float32)
    onesrep = np.zeros((128, 128), bf16)
    onesrep[0:64, 0:64] = 1
    onesrep[64:128, 64:128] = 1

    in_maps = []
    for c in range(N_CORES):
        rows = slice(DD * c, DD * (c + 1))

        def pack(w):  # [128 rows, C] -> packed lhsT chunks [128, 1024]
            chunks = [
                np.ascontiguousarray(w[:, kc * 128 : (kc + 1) * 128].T)
                for kc in range(C // 128)
            ]
            return np.concatenate(chunks, axis=1).astype(bf16)

        in_maps.append(
            {
                "xt": xt,
                "wq": pack(qkv_w[0 * C :][rows, :]),
                "wk": pack(qkv_w[1 * C + DD * c : 1 * C + DD * (c + 1), :]),
                "wv": pack(qkv_w[2 * C + DD * c : 2 * C + DD * (c + 1), :]),
                "pw": np.ascontiguousarray(proj_w[:, rows].T).astype(bf16),
                "qw": qw_col,
                "kw": kw_col,
                "onesrep": onesrep,
            }
        )
    return in_maps


def run_on_device(in_maps, reps=1, hw_loop=0):
    from concourse.bass_utils import run_bass_kernel_spmd

    key = (reps, hw_loop)
    if key not in _COMPILED:
        _COMPILED[key] = build_program(reps, hw_loop=hw_loop)
    nc = _COMPILED[key]
    res = run_bass_kernel_spmd(nc, in_maps, list(range(N_CORES)))
    return res


def kernel(x, qkv_w, q_norm_w, k_norm_w, proj_w, proj_b):
    x = np.asarray(x, np.float32)
    qkv_w = np.asarray(qkv_w, np.float32)
    proj_w = np.asarray(proj_w, np.float32)
    in_maps = _prepare_inputs(
        x, qkv_w, np.asarray(q_norm_w, np.float32), np.asarray(k_norm_w, np.float32), proj_w
    )
    res = run_on_device(in_maps, reps=1)
    y = np.zeros((B, N, C), np.float32)
    for c in range(N_CORES):
        y += np.asarray(res.results[c]["y"], dtype=np.float32)
    y += np.asarray(proj_b, np.float32)[None, None, :]
    return y
